# revision 23
# baseline (speedup 1.0000x reference)
"""Trainium2 Bass kernel for an additive-attention module.

Computes, for inputs s_tm1[B,DEC], xs_h[L,B,DENC], uh[L,B,ALIGN],
xs_mask[L,B], weights sa_w/sa_b/a1_w/a1_b:

    sa_s    = s_tm1 @ sa_w.T + sa_b                  # [B, ALIGN]
    tanh_sa = tanh(sa_s[None] + uh)                  # [L, B, ALIGN]
    a1      = einsum('lba,a->lb', tanh_sa, a1_w[0]) + a1_b[0]
    e_ij    = masked softmax of a1 over dim 0 (L)
    attend  = einsum('lb,lbd->bd', e_ij, xs_h)       # [B, DENC]

returning (tanh_sa, a1_w, a1, e_ij, attend).

Strategy: data-parallel over the batch dim across 8 NeuronCores
(8 batches per core); weights replicated.  Per core the kernel streams
uh in l-tiles of 128 rows laid out [128 partitions (l), (b, a) free],
adds the broadcast sa_s on VectorE, applies tanh on ScalarE (in place),
stores the tanh tile, and reduces (tanh * a1_w) over ALIGN on VectorE
to build a1.  a1 is transposed tile-wise on TensorE into a [b, L]
layout where the L-softmax is a plain free-dim reduction.  attend
contracts e * xs_h over L on TensorE: e columns are expanded into
one-hot-by-batch [128, 8] stationary operands so all batches accumulate
into a single [8, 1024] PSUM region.
"""

import functools
import sys

import numpy as np


def _ensure_import_path():
    try:
        import concourse.bass  # noqa: F401
        return
    except ImportError:
        pass
    for p in ("/opt/trn_rl_repo", "/root/.axon_site/_ro/trn_rl_repo"):
        if p not in sys.path:
            sys.path.append(p)
    import concourse.bass  # noqa: F401


L, B, DEC, ALIGN, DENC = 1024, 64, 512, 512, 1024
NCORES = 8
BL = B // NCORES          # 8 batches per core
T = L // 128              # 8 l-tiles per core


def build_nc(compile=True, loop_iters=None):
    """Build the Bass program.  loop_iters wraps the whole body in a
    device-side For_i loop — used only for timing (the per-execution RPC
    floor through axon is ~80ms, so HW time is measured by running the
    kernel K times in one NEFF execution)."""
    _ensure_import_path()
    from contextlib import ExitStack

    import concourse.bacc as bacc
    import concourse.mybir as mybir
    import concourse.tile as tile

    F32 = mybir.dt.float32
    F32R = mybir.dt.float32r
    AF = mybir.ActivationFunctionType
    ALU = mybir.AluOpType
    AX = mybir.AxisListType

    nc = bacc.Bacc("TRN2", debug=False, num_devices=NCORES)

    def din(name, shape):
        return nc.dram_tensor(name, shape, F32, kind="ExternalInput").ap()

    def dout(name, shape):
        return nc.dram_tensor(name, shape, F32, kind="ExternalOutput").ap()

    sT_ap = din("s_tm1T", [DEC, BL])            # s_tm1 shard, pre-transposed
    uh_ap = din("uh", [L, BL, ALIGN])
    xs_ap = din("xs_h", [L, BL, DENC])
    maskT_ap = din("maskT", [BL, L])            # xs_mask shard, pre-transposed
    saw_ap = din("sa_wT", [DEC, ALIGN])         # sa_w pre-transposed
    sab_ap = din("sa_b", [1, ALIGN])
    a1w_ap = din("a1_w", [1, ALIGN])
    a1b_ap = din("a1_b", [1, 1])
    id_ap = din("identity", [128, 128])         # np.eye(128)
    bc8_ap = din("bcast8", [BL, BL * 128])      # bcast8[k, b*128+p] = (k == b)

    tanh_ap = dout("tanh_sa", [L, BL, ALIGN])
    a1p_ap = dout("a1p", [128, T * BL])         # a1p[p, t*BL+b] = a1[t*128+p, b]
    ep_ap = dout("ep", [128, T * BL])           # same layout for e_ij
    att_ap = dout("attend", [BL, DENC])

    with tile.TileContext(nc) as tc:
        with ExitStack() as ctx:
            cpool = ctx.enter_context(tc.tile_pool(name="consts", bufs=1))
            work = ctx.enter_context(tc.tile_pool(name="work", bufs=3))
            xsp = ctx.enter_context(tc.tile_pool(name="xs", bufs=4))
            scr = ctx.enter_context(tc.tile_pool(name="scr", bufs=1))
            ps_set = ctx.enter_context(
                tc.tile_pool(name="ps_set", bufs=2, space="PSUM"))
            ps_tr = ctx.enter_context(
                tc.tile_pool(name="ps_tr", bufs=2, space="PSUM"))
            ps_att = ctx.enter_context(
                tc.tile_pool(name="ps_att", bufs=1, space="PSUM"))

            def body():
                # ---- constants / small inputs ----
                id_sb = cpool.tile([128, 128], F32)
                nc.scalar.dma_start(id_sb[:], id_ap[:])
                bc8 = cpool.tile([BL, BL * 128], F32)
                nc.scalar.dma_start(bc8[:], bc8_ap[:])
                maskT = cpool.tile([BL, L], F32)
                nc.scalar.dma_start(maskT[:], maskT_ap[:])
                sab = cpool.tile([1, ALIGN], F32)
                nc.scalar.dma_start(sab[:], sab_ap[:])
                a1w = cpool.tile([1, ALIGN], F32)
                nc.scalar.dma_start(a1w[:], a1w_ap[:])
                a1b = cpool.tile([1, 1], F32)
                nc.scalar.dma_start(a1b[:], a1b_ap[:])
                sT = cpool.tile([128, 4 * BL], F32)
                saw = cpool.tile([128, 4 * ALIGN], F32)
                for c in range(4):
                    nc.scalar.dma_start(sT[:, c * BL:(c + 1) * BL],
                                      sT_ap[c * 128:(c + 1) * 128])
                    nc.scalar.dma_start(saw[:, c * ALIGN:(c + 1) * ALIGN],
                                      saw_ap[c * 128:(c + 1) * 128])
                ones = cpool.tile([1, 128], F32)
                nc.vector.memset(ones[:], 1.0)

                # ---- sa_s = s_tm1 @ sa_w.T + sa_b  -> [BL, ALIGN] ----
                ps_s = ps_set.tile([BL, ALIGN], F32, tag="ps_set")
                for c in range(4):
                    nc.tensor.matmul(ps_s[:], sT[:, c * BL:(c + 1) * BL],
                                     saw[:, c * ALIGN:(c + 1) * ALIGN],
                                     start=(c == 0), stop=False)
                # + sa_b broadcast over b via a K=1 matmul with a ones row
                nc.tensor.matmul(ps_s[:], ones[:, :BL], sab[:],
                                 start=False, stop=True)
                sa_sb = cpool.tile([BL, ALIGN], F32)
                nc.scalar.copy(sa_sb[:], ps_s[:])

                # ---- partition broadcasts via ones/one-hot matmuls ----
                # a1w_rep[p, a] = a1_w[a]
                a1w_rep = cpool.tile([128, ALIGN], F32)
                pb = ps_set.tile([128, ALIGN], F32, tag="ps_set")
                nc.tensor.matmul(pb[:], ones[:], a1w[:], start=True, stop=True)
                nc.scalar.copy(a1w_rep[:], pb[:])

                a1b_rep = cpool.tile([128, 1], F32)
                pb2 = ps_set.tile([128, 1], F32, tag="ps_set")
                nc.tensor.matmul(pb2[:], ones[:], a1b[:], start=True,
                                 stop=True)
                nc.scalar.copy(a1b_rep[:], pb2[:])

                # sa_rep[p, b*ALIGN + a] = sa_s[b, a] for every partition p
                sa_rep = cpool.tile([128, BL * ALIGN], F32)
                for b in range(BL):
                    pbc = ps_set.tile([128, ALIGN], F32, tag="ps_set")
                    nc.tensor.matmul(pbc[:], bc8[:, b * 128:(b + 1) * 128],
                                     sa_sb[:], start=True, stop=True)
                    nc.scalar.copy(sa_rep[:, b * ALIGN:(b + 1) * ALIGN],
                                   pbc[:])

                # ---- phase A: stream uh, tanh, a1 ----
                a1_full = cpool.tile([128, T * BL], F32)
                a1T = cpool.tile([BL, L], F32)
                eT = cpool.tile([BL, L], F32)
                psum_t = cpool.tile([BL, T], F32)
                xs_tiles = []
                for t in range(T):
                    w = work.tile([128, BL * ALIGN], F32, tag="work")
                    nc.sync.dma_start(
                        w[:].rearrange("p (b a) -> p b a", b=BL),
                        uh_ap[t * 128:(t + 1) * 128])
                    # xs in two 2MB halves on the SWDGE ring (own DMA queue,
                    # never blocks the uh loads on the SP ring); written as
                    # f32r so the attend matmuls run single-pass on the PE
                    halves = []
                    for h in range(2):
                        xh = xsp.tile([128, BL * DENC // 2], F32R, tag="xs")
                        nc.gpsimd.dma_start(
                            xh[:].rearrange("p (b d) -> p b d", b=BL // 2),
                            xs_ap[t * 128:(t + 1) * 128,
                                  h * (BL // 2):(h + 1) * (BL // 2)])
                        halves.append(xh)
                    xs_tiles.append(halves)

                    nc.vector.tensor_add(w[:], w[:], sa_rep[:])
                    nc.scalar.activation(w[:], w[:], AF.Tanh)
                    # store via the ACT HWDGE ring to split DMA issue rings
                    nc.scalar.dma_start(
                        tanh_ap[t * 128:(t + 1) * 128],
                        w[:].rearrange("p (b a) -> p b a", b=BL))

                    # a1 tile: sum over ALIGN of tanh * a1_w, fused as
                    # out=(tanh bypass 0) * a1w, accum=sum(out)
                    # (tensor_tensor_reduce is fatal on this runtime: NRT 101)
                    sc = scr.tile([128, ALIGN], F32, tag="scr")
                    for b in range(BL):
                        nc.vector.scalar_tensor_tensor(
                            out=sc[:],
                            in0=w[:, b * ALIGN:(b + 1) * ALIGN],
                            scalar=0.0,
                            in1=a1w_rep[:],
                            op0=ALU.bypass,
                            op1=ALU.mult,
                            accum_out=a1_full[:, t * BL + b:t * BL + b + 1])

                    pt = ps_tr.tile([BL, 128], F32, tag="ps_tr")
                    nc.tensor.transpose(pt[:],
                                        a1_full[:, t * BL:(t + 1) * BL],
                                        id_sb[:])
                    nc.vector.tensor_copy(a1T[:, t * 128:(t + 1) * 128],
                                          pt[:])
                    # softmax is shift-invariant and |a1| <= sum|a1_w| < ~10,
                    # so exp needs no max subtraction -> exp, mask and the
                    # partial sums all run per-tile, overlapped with phase A
                    sl = slice(t * 128, (t + 1) * 128)
                    nc.scalar.activation(eT[:, sl], a1T[:, sl], AF.Exp)
                    nc.vector.tensor_mul(eT[:, sl], eT[:, sl], maskT[:, sl])
                    nc.vector.reduce_sum(psum_t[:, t:t + 1], eT[:, sl],
                                         axis=AX.X)

                # ---- normalize ----
                ssum = cpool.tile([BL, 1], F32)
                nc.vector.reduce_sum(ssum[:], psum_t[:], axis=AX.X)
                rec = cpool.tile([BL, 1], F32)
                nc.vector.reciprocal(rec[:], ssum[:])
                enorm = cpool.tile([BL, L], F32)
                nc.vector.tensor_scalar_mul(enorm[:], eT[:], rec[:])

                # a1 output = a1_full + a1_b
                a1o = cpool.tile([128, T * BL], F32)
                nc.scalar.activation(a1o[:], a1_full[:], AF.Identity,
                                     bias=a1b_rep[:], scale=1.0)
                nc.sync.dma_start(a1p_ap[:], a1o[:])

                # ---- transpose e back to [128, (t, b)], one-hot expand ----
                e_full = cpool.tile([128, T * BL], F32)
                for t in range(T):
                    pe = ps_tr.tile([128, BL], F32, tag="ps_tr2")
                    nc.tensor.transpose(pe[:],
                                        enorm[:, t * 128:(t + 1) * 128],
                                        id_sb[:BL, :BL])
                    nc.vector.tensor_copy(e_full[:, t * BL:(t + 1) * BL],
                                          pe[:])
                nc.sync.dma_start(ep_ap[:], e_full[:])

                # e_exp[p, t*64 + b*8 + b'] = e[t*128+p, b] if b'==b else 0
                # (built in f32 — Memset can't write f32r — then cast-copied
                # to f32r, which the fp32r matmuls require of their producer)
                e_exp = cpool.tile([128, T * BL * BL], F32)
                nc.vector.memset(e_exp[:], 0.0)
                diag = e_exp[:].rearrange(
                    "p (t x) -> p t x", t=T)[:, :, 0:BL * BL:BL + 1]
                nc.vector.tensor_copy(
                    diag, e_full[:].rearrange("p (t b) -> p t b", t=T))
                e_expr = cpool.tile([128, T * BL * BL], F32R)
                nc.vector.tensor_copy(e_expr[:], e_exp[:])

                # ---- attend = sum_l e[l, b] * xs_h[l, b, :] ----
                attp = ps_att.tile([BL, DENC], F32)
                for t in range(T):
                    for b in range(BL):
                        xh = xs_tiles[t][b // (BL // 2)]
                        bh = b % (BL // 2)
                        lhs = e_expr[:, t * 64 + b * BL:t * 64 + (b + 1) * BL]
                        for j in range(2):
                            # fp32r: single-pass PE (4x the fp32 rate)
                            nc.tensor.matmul(
                                attp[:, j * 512:(j + 1) * 512],
                                lhs,
                                xh[:, bh * DENC + j * 512:
                                   bh * DENC + (j + 1) * 512],
                                start=(t == 0 and b == 0),
                                stop=(t == T - 1 and b == BL - 1),
                                skip_group_check=True)
                att_sb = cpool.tile([BL, DENC], F32)
                nc.scalar.copy(att_sb[:], attp[:])
                nc.sync.dma_start(att_ap[:], att_sb[:])

            if loop_iters is None:
                body()
            else:
                with tc.For_i(0, loop_iters, 1):
                    body()

    if compile:
        nc.compile()
    return nc


@functools.lru_cache(maxsize=1)
def _get_nc():
    return build_nc()


def make_in_maps(s_tm1, xs_h, uh, xs_mask, sa_w, sa_b, a1_w, a1_b):
    f = np.float32
    s_tm1 = np.asarray(s_tm1, f)
    xs_h = np.asarray(xs_h, f)
    uh = np.asarray(uh, f)
    xs_mask = np.asarray(xs_mask, f)
    sa_w = np.asarray(sa_w, f)
    sa_b = np.asarray(sa_b, f)
    a1_w = np.asarray(a1_w, f)
    a1_b = np.asarray(a1_b, f)

    identity = np.eye(128, dtype=f)
    bcast8 = np.repeat(np.eye(BL, dtype=f), 128, axis=1)
    saT = np.ascontiguousarray(sa_w.T)
    sab = np.ascontiguousarray(sa_b.reshape(1, ALIGN))
    a1w = np.ascontiguousarray(a1_w.reshape(1, ALIGN))
    a1b = np.ascontiguousarray(a1_b.reshape(1, 1))

    in_maps = []
    for i in range(NCORES):
        bs = slice(i * BL, (i + 1) * BL)
        in_maps.append({
            "s_tm1T": np.ascontiguousarray(s_tm1[bs].T),
            "uh": np.ascontiguousarray(uh[:, bs, :]),
            "xs_h": np.ascontiguousarray(xs_h[:, bs, :]),
            "maskT": np.ascontiguousarray(xs_mask[:, bs].T),
            "sa_wT": saT,
            "sa_b": sab,
            "a1_w": a1w,
            "a1_b": a1b,
            "identity": identity,
            "bcast8": bcast8,
        })
    return in_maps, a1w


def _unpack_col(x):
    # [128, T*BL] with cols (t, b) -> [L, BL]
    return np.ascontiguousarray(
        x.reshape(128, T, BL).transpose(1, 0, 2).reshape(L, BL))


def gather_outputs(results, a1_w_full):
    tanh_sa = np.concatenate([r["tanh_sa"] for r in results], axis=1)
    a1 = np.concatenate([_unpack_col(r["a1p"]) for r in results], axis=1)
    e_ij = np.concatenate([_unpack_col(r["ep"]) for r in results], axis=1)
    attend = np.concatenate([r["attend"] for r in results], axis=0)
    return (tanh_sa, a1_w_full, a1, e_ij, attend)


def run(trace=False, **inputs):
    _ensure_import_path()
    from concourse.bass_utils import run_bass_kernel_spmd

    in_maps, _ = make_in_maps(**inputs)
    nc = _get_nc()
    res = run_bass_kernel_spmd(nc, in_maps, list(range(NCORES)), trace=trace)
    out = gather_outputs(res.results, np.asarray(inputs["a1_w"], np.float32))
    return out, res


def kernel(s_tm1, xs_h, uh, xs_mask, sa_w, sa_b, a1_w, a1_b):
    out, _ = run(s_tm1=s_tm1, xs_h=xs_h, uh=uh, xs_mask=xs_mask,
                 sa_w=sa_w, sa_b=sa_b, a1_w=a1_w, a1_b=a1_b)
    return out


# revision 26
# speedup vs baseline: 1.6974x; 1.6974x over previous
"""Trainium2 Bass kernel for an additive-attention module.

Computes, for inputs s_tm1[B,DEC], xs_h[L,B,DENC], uh[L,B,ALIGN],
xs_mask[L,B], weights sa_w/sa_b/a1_w/a1_b:

    sa_s    = s_tm1 @ sa_w.T + sa_b                  # [B, ALIGN]
    tanh_sa = tanh(sa_s[None] + uh)                  # [L, B, ALIGN]
    a1      = einsum('lba,a->lb', tanh_sa, a1_w[0]) + a1_b[0]
    e_ij    = masked softmax of a1 over dim 0 (L)
    attend  = einsum('lb,lbd->bd', e_ij, xs_h)       # [B, DENC]

returning (tanh_sa, a1_w, a1, e_ij, attend).

Strategy: data-parallel over the batch dim across 8 NeuronCores
(8 batches per core); weights replicated.  Per core the kernel streams
uh in l-tiles of 128 rows laid out [128 partitions (l), (b, a) free],
adds the broadcast sa_s on VectorE, applies tanh on ScalarE (in place),
stores the tanh tile, and reduces (tanh * a1_w) over ALIGN on VectorE
to build a1.  a1 is transposed tile-wise on TensorE into a [b, L]
layout where the L-softmax is a plain free-dim reduction.  attend
contracts e * xs_h over L on TensorE: e columns are expanded into
one-hot-by-batch [128, 8] stationary operands so all batches accumulate
into a single [8, 1024] PSUM region.
"""

import functools
import sys

import numpy as np


def _ensure_import_path():
    try:
        import concourse.bass  # noqa: F401
        return
    except ImportError:
        pass
    for p in ("/opt/trn_rl_repo", "/root/.axon_site/_ro/trn_rl_repo"):
        if p not in sys.path:
            sys.path.append(p)
    import concourse.bass  # noqa: F401


L, B, DEC, ALIGN, DENC = 1024, 64, 512, 512, 1024
NCORES = 8
BL = B // NCORES          # 8 batches per core
T = L // 128              # 8 l-tiles per core


def build_nc(compile=True, loop_iters=None):
    """Build the Bass program.  loop_iters wraps the whole body in a
    device-side For_i loop — used only for timing (the per-execution RPC
    floor through axon is ~80ms, so HW time is measured by running the
    kernel K times in one NEFF execution)."""
    _ensure_import_path()
    from contextlib import ExitStack

    import concourse.bacc as bacc
    import concourse.mybir as mybir
    import concourse.tile as tile

    F32 = mybir.dt.float32
    F32R = mybir.dt.float32r
    AF = mybir.ActivationFunctionType
    ALU = mybir.AluOpType
    AX = mybir.AxisListType

    nc = bacc.Bacc("TRN2", debug=False, num_devices=NCORES)

    def din(name, shape):
        return nc.dram_tensor(name, shape, F32, kind="ExternalInput").ap()

    def dout(name, shape):
        return nc.dram_tensor(name, shape, F32, kind="ExternalOutput").ap()

    sT_ap = din("s_tm1T", [DEC, BL])            # s_tm1 shard, pre-transposed
    uh_ap = din("uh", [L, BL, ALIGN])
    xs_ap = din("xs_h", [L, BL, DENC])
    maskT_ap = din("maskT", [BL, L])            # xs_mask shard, pre-transposed
    saw_ap = din("sa_wT", [DEC, ALIGN])         # sa_w pre-transposed
    sab_ap = din("sa_b", [1, ALIGN])
    a1w_ap = din("a1_w", [1, ALIGN])
    a1b_ap = din("a1_b", [1, 1])
    id_ap = din("identity", [128, 128])         # np.eye(128)
    bc8_ap = din("bcast8", [BL, BL * 128])      # bcast8[k, b*128+p] = (k == b)

    tanh_ap = dout("tanh_sa", [L, BL, ALIGN])
    a1p_ap = dout("a1p", [128, T * BL])         # a1p[p, t*BL+b] = a1[t*128+p, b]
    ep_ap = dout("ep", [128, T * BL])           # same layout for e_ij
    att_ap = dout("attend", [BL, DENC])

    with tile.TileContext(nc) as tc:
        with ExitStack() as ctx:
            cpool = ctx.enter_context(tc.tile_pool(name="consts", bufs=1))
            work = ctx.enter_context(tc.tile_pool(name="work", bufs=3))
            xsp = ctx.enter_context(tc.tile_pool(name="xs", bufs=4))
            scr = ctx.enter_context(tc.tile_pool(name="scr", bufs=1))
            ps_set = ctx.enter_context(
                tc.tile_pool(name="ps_set", bufs=2, space="PSUM"))
            ps_tr = ctx.enter_context(
                tc.tile_pool(name="ps_tr", bufs=2, space="PSUM"))
            ps_att = ctx.enter_context(
                tc.tile_pool(name="ps_att", bufs=1, space="PSUM"))

            def body():
                # ---- constants / small inputs ----
                # all on the ACT HWDGE ring, gating tensors (sT/saw) first;
                # the t=0 streaming loads take an explicit dep on the last of
                # these so ~60KB of setup never queues behind 2MB transfers
                sT = cpool.tile([128, 4 * BL], F32)
                saw = cpool.tile([128, 4 * ALIGN], F32)
                for c in range(4):
                    nc.scalar.dma_start(sT[:, c * BL:(c + 1) * BL],
                                        sT_ap[c * 128:(c + 1) * 128])
                    nc.scalar.dma_start(saw[:, c * ALIGN:(c + 1) * ALIGN],
                                        saw_ap[c * 128:(c + 1) * 128])
                sab = cpool.tile([1, ALIGN], F32)
                nc.scalar.dma_start(sab[:], sab_ap[:])
                bc8 = cpool.tile([BL, BL * 128], F32)
                nc.scalar.dma_start(bc8[:], bc8_ap[:])
                id_sb = cpool.tile([128, 128], F32)
                nc.scalar.dma_start(id_sb[:], id_ap[:])
                maskT = cpool.tile([BL, L], F32)
                nc.scalar.dma_start(maskT[:], maskT_ap[:])
                a1w = cpool.tile([1, ALIGN], F32)
                nc.scalar.dma_start(a1w[:], a1w_ap[:])
                a1b = cpool.tile([1, 1], F32)
                last_const = nc.scalar.dma_start(a1b[:], a1b_ap[:])
                ones = cpool.tile([1, 128], F32)
                nc.vector.memset(ones[:], 1.0)

                # ---- sa_s = s_tm1 @ sa_w.T + sa_b  -> [BL, ALIGN] ----
                ps_s = ps_set.tile([BL, ALIGN], F32, tag="ps_set")
                for c in range(4):
                    nc.tensor.matmul(ps_s[:], sT[:, c * BL:(c + 1) * BL],
                                     saw[:, c * ALIGN:(c + 1) * ALIGN],
                                     start=(c == 0), stop=False)
                # + sa_b broadcast over b via a K=1 matmul with a ones row
                nc.tensor.matmul(ps_s[:], ones[:, :BL], sab[:],
                                 start=False, stop=True)
                sa_sb = cpool.tile([BL, ALIGN], F32)
                nc.scalar.copy(sa_sb[:], ps_s[:])

                # ---- partition broadcasts via ones/one-hot matmuls ----
                # a1w_rep[p, a] = a1_w[a]
                a1w_rep = cpool.tile([128, ALIGN], F32)
                pb = ps_set.tile([128, ALIGN], F32, tag="ps_set")
                nc.tensor.matmul(pb[:], ones[:], a1w[:], start=True, stop=True)
                nc.scalar.copy(a1w_rep[:], pb[:])

                a1b_rep = cpool.tile([128, 1], F32)
                pb2 = ps_set.tile([128, 1], F32, tag="ps_set")
                nc.tensor.matmul(pb2[:], ones[:], a1b[:], start=True,
                                 stop=True)
                nc.scalar.copy(a1b_rep[:], pb2[:])

                # sa_rep[p, b*ALIGN + a] = sa_s[b, a] for every partition p
                sa_rep = cpool.tile([128, BL * ALIGN], F32)
                for b in range(BL):
                    pbc = ps_set.tile([128, ALIGN], F32, tag="ps_set")
                    nc.tensor.matmul(pbc[:], bc8[:, b * 128:(b + 1) * 128],
                                     sa_sb[:], start=True, stop=True)
                    nc.scalar.copy(sa_rep[:, b * ALIGN:(b + 1) * ALIGN],
                                   pbc[:])

                # ---- phase A: stream uh, tanh, a1 ----
                a1_full = cpool.tile([128, T * BL], F32)
                a1T = cpool.tile([BL, L], F32)
                eT = cpool.tile([BL, L], F32)
                psum_t = cpool.tile([BL, T], F32)
                xs_tiles = []
                for t in range(T):
                    w = work.tile([128, BL * ALIGN], F32, tag="work")
                    i_uh = nc.sync.dma_start(
                        w[:].rearrange("p (b a) -> p b a", b=BL),
                        uh_ap[t * 128:(t + 1) * 128])
                    if t == 0:
                        tile.add_dep_helper(i_uh.ins, last_const.ins,
                                            reason="setup DMAs first")
                    # xs in two 2MB halves on the SWDGE ring (own DMA queue,
                    # never blocks the uh loads on the SP ring); written as
                    # f32r so the attend matmuls run single-pass on the PE
                    halves = []
                    for h in range(2):
                        xh = xsp.tile([128, BL * DENC // 2], F32R, tag="xs")
                        i_xs = nc.gpsimd.dma_start(
                            xh[:].rearrange("p (b d) -> p b d", b=BL // 2),
                            xs_ap[t * 128:(t + 1) * 128,
                                  h * (BL // 2):(h + 1) * (BL // 2)])
                        if t == 0:
                            tile.add_dep_helper(i_xs.ins, last_const.ins,
                                                reason="setup DMAs first")
                        halves.append(xh)
                    xs_tiles.append(halves)

                    nc.vector.tensor_add(w[:], w[:], sa_rep[:])
                    nc.scalar.activation(w[:], w[:], AF.Tanh)
                    # store via the ACT HWDGE ring to split DMA issue rings
                    nc.scalar.dma_start(
                        tanh_ap[t * 128:(t + 1) * 128],
                        w[:].rearrange("p (b a) -> p b a", b=BL))

                    # a1 tile: sum over ALIGN of tanh * a1_w, fused as
                    # out=(tanh bypass 0) * a1w, accum=sum(out)
                    # (tensor_tensor_reduce is fatal on this runtime: NRT 101)
                    sc = scr.tile([128, ALIGN], F32, tag="scr")
                    for b in range(BL):
                        nc.vector.scalar_tensor_tensor(
                            out=sc[:],
                            in0=w[:, b * ALIGN:(b + 1) * ALIGN],
                            scalar=0.0,
                            in1=a1w_rep[:],
                            op0=ALU.bypass,
                            op1=ALU.mult,
                            accum_out=a1_full[:, t * BL + b:t * BL + b + 1])

                    pt = ps_tr.tile([BL, 128], F32, tag="ps_tr")
                    nc.tensor.transpose(pt[:],
                                        a1_full[:, t * BL:(t + 1) * BL],
                                        id_sb[:])
                    nc.vector.tensor_copy(a1T[:, t * 128:(t + 1) * 128],
                                          pt[:])
                    # softmax is shift-invariant and |a1| <= sum|a1_w| < ~10,
                    # so exp needs no max subtraction -> exp, mask and the
                    # partial sums all run per-tile, overlapped with phase A
                    sl = slice(t * 128, (t + 1) * 128)
                    nc.scalar.activation(eT[:, sl], a1T[:, sl], AF.Exp)
                    nc.vector.tensor_mul(eT[:, sl], eT[:, sl], maskT[:, sl])
                    nc.vector.reduce_sum(psum_t[:, t:t + 1], eT[:, sl],
                                         axis=AX.X)

                # ---- normalize ----
                ssum = cpool.tile([BL, 1], F32)
                nc.vector.reduce_sum(ssum[:], psum_t[:], axis=AX.X)
                rec = cpool.tile([BL, 1], F32)
                nc.vector.reciprocal(rec[:], ssum[:])
                enorm = cpool.tile([BL, L], F32)
                nc.vector.tensor_scalar_mul(enorm[:], eT[:], rec[:])

                # a1 output = a1_full + a1_b
                a1o = cpool.tile([128, T * BL], F32)
                nc.scalar.activation(a1o[:], a1_full[:], AF.Identity,
                                     bias=a1b_rep[:], scale=1.0)
                nc.sync.dma_start(a1p_ap[:], a1o[:])

                # ---- transpose e back to [128, (t, b)], one-hot expand ----
                e_full = cpool.tile([128, T * BL], F32)
                for t in range(T):
                    pe = ps_tr.tile([128, BL], F32, tag="ps_tr2")
                    nc.tensor.transpose(pe[:],
                                        enorm[:, t * 128:(t + 1) * 128],
                                        id_sb[:BL, :BL])
                    nc.vector.tensor_copy(e_full[:, t * BL:(t + 1) * BL],
                                          pe[:])
                nc.sync.dma_start(ep_ap[:], e_full[:])

                # e_exp[p, t*64 + b*8 + b'] = e[t*128+p, b] if b'==b else 0
                # (built in f32 — Memset can't write f32r — then cast-copied
                # to f32r, which the fp32r matmuls require of their producer)
                e_exp = cpool.tile([128, T * BL * BL], F32)
                nc.vector.memset(e_exp[:], 0.0)
                diag = e_exp[:].rearrange(
                    "p (t x) -> p t x", t=T)[:, :, 0:BL * BL:BL + 1]
                nc.vector.tensor_copy(
                    diag, e_full[:].rearrange("p (t b) -> p t b", t=T))
                e_expr = cpool.tile([128, T * BL * BL], F32R)
                nc.vector.tensor_copy(e_expr[:], e_exp[:])

                # ---- attend = sum_l e[l, b] * xs_h[l, b, :] ----
                attp = ps_att.tile([BL, DENC], F32)
                for t in range(T):
                    for b in range(BL):
                        xh = xs_tiles[t][b // (BL // 2)]
                        bh = b % (BL // 2)
                        lhs = e_expr[:, t * 64 + b * BL:t * 64 + (b + 1) * BL]
                        for j in range(2):
                            # fp32r: single-pass PE (4x the fp32 rate)
                            nc.tensor.matmul(
                                attp[:, j * 512:(j + 1) * 512],
                                lhs,
                                xh[:, bh * DENC + j * 512:
                                   bh * DENC + (j + 1) * 512],
                                start=(t == 0 and b == 0),
                                stop=(t == T - 1 and b == BL - 1),
                                skip_group_check=True)
                att_sb = cpool.tile([BL, DENC], F32)
                nc.scalar.copy(att_sb[:], attp[:])
                nc.sync.dma_start(att_ap[:], att_sb[:])

            if loop_iters is None:
                body()
            else:
                with tc.For_i(0, loop_iters, 1):
                    body()

    if compile:
        nc.compile()
    return nc


@functools.lru_cache(maxsize=1)
def _get_nc():
    return build_nc()


def make_in_maps(s_tm1, xs_h, uh, xs_mask, sa_w, sa_b, a1_w, a1_b):
    f = np.float32
    s_tm1 = np.asarray(s_tm1, f)
    xs_h = np.asarray(xs_h, f)
    uh = np.asarray(uh, f)
    xs_mask = np.asarray(xs_mask, f)
    sa_w = np.asarray(sa_w, f)
    sa_b = np.asarray(sa_b, f)
    a1_w = np.asarray(a1_w, f)
    a1_b = np.asarray(a1_b, f)

    identity = np.eye(128, dtype=f)
    bcast8 = np.repeat(np.eye(BL, dtype=f), 128, axis=1)
    saT = np.ascontiguousarray(sa_w.T)
    sab = np.ascontiguousarray(sa_b.reshape(1, ALIGN))
    a1w = np.ascontiguousarray(a1_w.reshape(1, ALIGN))
    a1b = np.ascontiguousarray(a1_b.reshape(1, 1))

    in_maps = []
    for i in range(NCORES):
        bs = slice(i * BL, (i + 1) * BL)
        in_maps.append({
            "s_tm1T": np.ascontiguousarray(s_tm1[bs].T),
            "uh": np.ascontiguousarray(uh[:, bs, :]),
            "xs_h": np.ascontiguousarray(xs_h[:, bs, :]),
            "maskT": np.ascontiguousarray(xs_mask[:, bs].T),
            "sa_wT": saT,
            "sa_b": sab,
            "a1_w": a1w,
            "a1_b": a1b,
            "identity": identity,
            "bcast8": bcast8,
        })
    return in_maps, a1w


def _unpack_col(x):
    # [128, T*BL] with cols (t, b) -> [L, BL]
    return np.ascontiguousarray(
        x.reshape(128, T, BL).transpose(1, 0, 2).reshape(L, BL))


def gather_outputs(results, a1_w_full):
    tanh_sa = np.concatenate([r["tanh_sa"] for r in results], axis=1)
    a1 = np.concatenate([_unpack_col(r["a1p"]) for r in results], axis=1)
    e_ij = np.concatenate([_unpack_col(r["ep"]) for r in results], axis=1)
    attend = np.concatenate([r["attend"] for r in results], axis=0)
    return (tanh_sa, a1_w_full, a1, e_ij, attend)


def run(trace=False, **inputs):
    _ensure_import_path()
    from concourse.bass_utils import run_bass_kernel_spmd

    in_maps, _ = make_in_maps(**inputs)
    nc = _get_nc()
    res = run_bass_kernel_spmd(nc, in_maps, list(range(NCORES)), trace=trace)
    out = gather_outputs(res.results, np.asarray(inputs["a1_w"], np.float32))
    return out, res


def kernel(s_tm1, xs_h, uh, xs_mask, sa_w, sa_b, a1_w, a1_b):
    out, _ = run(s_tm1=s_tm1, xs_h=xs_h, uh=uh, xs_mask=xs_mask,
                 sa_w=sa_w, sa_b=sa_b, a1_w=a1_w, a1_b=a1_b)
    return out
